# revision 34
# baseline (speedup 1.0000x reference)
"""Trainium2 Bass kernel for nn_FMA_15427522527280 (sparse_attention).

Math (B=4, L=1024, D=4096):
  Q = x@wq.T + bq ; K = x@wk.T + bk ; V = x@wv.T + bv
  out0 = softmax(Q K^T / sqrt(D)) @ V
  Level-1: softmax over a SINGLE key => s1 == 1.0 exactly, so
  out1 = V1 = depthwise_conv(V, cvw, cvb) broadcast over seq.
  out = out0 + out1

Exact simplifications:
  - logits = Q K^T = x (wq^T wk) x^T + 1_q (bq wk) x^T  (+ terms that are
    per-query constants over keys, which softmax drops).  A = wq^T wk is
    precomputed on the host => the K projection GEMM disappears, and the
    Q projection becomes T = x @ A + 1 (bq wk).
  - bv & cvb fold into a host-side per-feature constant:
      host_add[d] = bv[d]*(1 + sum_k cvw[d,k]) + cvb[d]
    (softmax rows sum to 1), device computes
      S@V0 + colsum_k(cvw[d,k]*V0[k,d])   with V0 = x@wv.T

Numerics: all GEMMs bf16 (measured rel-err ~3.9e-3 vs 2e-2 budget;
fp8/DoubleRow was tried and rejected: 2.1e-2 on the real data);
accumulation fp32 in PSUM; final out fp32.

Sharding: 8 cores = 4 batches x 2 query-halves.  The V projection is
split over the pair by output-feature half and exchanged with per-slice
HBM AllGathers (replica groups {2b, 2b+1}) that overlap the remaining
V compute; everything else is per-core.

Phases per core (xT resident in SBUF as bf16 throughout):
  V:  V0[k, d-half] = xT.T @ wvT(half)  -> DRAM, AllGather -> full V0
  T:  TT[d,q] = A^T @ xTq + u           -> SBUF resident (bf16)
  EW: ew_r[ds] = sum_kb cvw.T*V0        (vector engine, under T)
  L:  logits -> softmax (no max-sub; logits*scale ~ N(0,1)) -> P^T
  O:  out = P^T.T @ V0 + ones*ew_r
"""

import numpy as np

P = 128
ASCL = 64.0


def _cfg(D, L, QH):
    assert D % 512 == 0 and L % P == 0 and QH % P == 0
    EB = D // P
    cfg = dict(
        D=D, L=L, QH=QH,
        EB=EB,                 # input-feature blocks (contraction)
        DB=D // P,             # T feature blocks
        DGN=D // 512,          # 512-wide output groups for T
        QS=QH // P,            # query subtiles
        KB=L // P,             # key blocks
        NL=min(512, L),        # logits N tile
        NDS=D // 512,          # 512-wide d slices for V/out
        ECW=min(8, EB),        # wv chunk width (e-blocks per chunk)
    )
    cfg["KN"] = L // cfg["NL"]
    cfg["ECN"] = EB // cfg["ECW"]
    cfg["NDSH"] = cfg["NDS"] // 2   # V d-slices computed per core
    assert EB % cfg["ECW"] == 0
    assert cfg["KB"] <= 8, "V accumulators use one PSUM bank per key block"
    return cfg


def build(cfg):
    from concourse import bacc
    import concourse.mybir as mybir
    import concourse.tile as tile
    from concourse.masks import make_identity

    f32 = mybir.dt.float32
    f32r = mybir.dt.float32r
    bf16 = mybir.dt.bfloat16
    fp8 = mybir.dt.float8e4
    DR = mybir.MatmulPerfMode.DoubleRow
    Ident = mybir.ActivationFunctionType.Identity
    Exp = mybir.ActivationFunctionType.Exp

    D, L, QH = cfg["D"], cfg["L"], cfg["QH"]
    EB, DB, DGN = cfg["EB"], cfg["DB"], cfg["DGN"]
    QS, KB, NL, KN = cfg["QS"], cfg["KB"], cfg["NL"], cfg["KN"]
    NDS, ECW, ECN, NDSH = cfg["NDS"], cfg["ECW"], cfg["ECN"], cfg["NDSH"]
    EBH = EB // 2
    scale = 1.0 / float(np.sqrt(D))

    nc = bacc.Bacc("TRN2", target_bir_lowering=False)

    xT = nc.dram_tensor("xT", [D, L], bf16, kind="ExternalInput")
    xTq = nc.dram_tensor("xTq", [D, QH], bf16, kind="ExternalInput")
    Ah = nc.dram_tensor("Ah", [EB, DGN, P, 512], bf16, kind="ExternalInput")
    wvT = nc.dram_tensor("wvT", [NDSH, ECN, P, ECW, 512], bf16,
                         kind="ExternalInput")
    cvT = nc.dram_tensor("cvT", [NDS, P, KB, 512], bf16, kind="ExternalInput")
    uh = nc.dram_tensor("uh", [P, DB], f32, kind="ExternalInput")
    onesd = nc.dram_tensor("onesd", [P, P], f32r, kind="ExternalInput")
    out = nc.dram_tensor("out", [QH, D], f32, kind="ExternalOutput")

    v_loc = nc.dram_tensor("v_loc", [NDSH, KB, P, 512], bf16)
    # ds-major so each ds-slice can be gathered as soon as it is computed
    v_gth = nc.dram_tensor("v_gth", [NDSH, 2, KB, P, 512], bf16)
    rgroups = [[0, 1], [2, 3], [4, 5], [6, 7]]

    with tile.TileContext(nc) as tc:
        with tc.tile_pool(name="const", bufs=1) as constp:
            ones = constp.tile([P, P], f32r, tag="ones", name="ones")
            nc.sync.dma_start(ones[:], onesd[:])
            u_sb = constp.tile([P, DB], f32, tag="usb", name="u_sb")
            nc.sync.dma_start(u_sb[:], uh[:])
            ident = constp.tile([P, P], bf16, tag="ident", name="ident")
            make_identity(nc, ident)

            with (
                tc.tile_pool(name="xt", bufs=1) as xtp,
                tc.tile_pool(name="tt", bufs=1) as ttp,
                tc.tile_pool(name="ptp", bufs=1) as ptp,
            ):
                EBL = EB // 2
                xt_lo = xtp.tile([P, EBL, L], bf16, tag="xtl", name="xt_lo")
                xt_hi = xtp.tile([P, EBL, L], bf16, tag="xth", name="xt_hi")
                for eb in range(EB):
                    dst = xt_lo if eb < EBL else xt_hi
                    nc.sync.dma_start(dst[:, eb % EBL, :],
                                      xT[eb * P:(eb + 1) * P, :])

                def xte(eb):
                    return (xt_lo if eb < EBL else xt_hi)[:, eb % EBL]

                xtq = xtp.tile([P, EB, QH], bf16, tag="xtq", name="xtq")
                tt = ttp.tile([P, DB, QH], bf16, tag="tt", name="tt")
                pt_sb = ptp.tile([P, KB, QH], bf16, tag="pt", name="pt_sb")

                # --- V: V0[k, d-half] = x @ wv^T(half) -> AllGather -------
                with (
                    tc.tile_pool(name="wv", bufs=3) as wvp,
                    tc.tile_pool(name="vcb", bufs=6) as vcb,
                    tc.tile_pool(name="psv", bufs=8, space="PSUM") as psvp,
                ):
                    for ds in range(NDSH):
                        psv = [psvp.tile([P, 512], f32, tag="ps",
                                         name=f"psv_{ds}_{kb}")
                               for kb in range(KB)]
                        for ec in range(ECN):
                            wc = wvp.tile([P, ECW, 512], bf16, tag="wv",
                                          name=f"wv_{ds}_{ec}")
                            nc.sync.dma_start(wc[:], wvT[ds, ec])
                            for j in range(ECW):
                                eb = ec * ECW + j
                                for kb in range(KB):
                                    nc.tensor.matmul(
                                        psv[kb][:],
                                        xte(eb)[:, kb * P:(kb + 1) * P],
                                        wc[:, j, :],
                                        start=(eb == 0), stop=(eb == EB - 1))
                        for kb in range(KB):
                            vsb = vcb.tile([P, 512], bf16, tag="v",
                                           name=f"v_{ds}_{kb}")
                            nc.vector.tensor_copy(vsb[:], psv[kb][:])
                            nc.sync.dma_start(v_loc[ds, kb], vsb[:])
                        # gather this slice while the next one computes
                        nc.gpsimd.collective_compute(
                            "AllGather", mybir.AluOpType.bypass,
                            replica_groups=rgroups,
                            ins=[v_loc[ds].opt()],
                            outs=[v_gth[ds].opt()])

                # --- T: TT[d,q] = A^T @ xq + u  (bf16) --------------------
                with (
                    tc.tile_pool(name="w1", bufs=10) as w1p,
                    tc.tile_pool(name="ps1", bufs=8, space="PSUM") as ps1,
                ):
                    # issued here so these DMAs queue behind the V-phase's
                    # xt/wv loads rather than ahead of them
                    for eb in range(EB):
                        nc.sync.dma_start(xtq[:, eb, :],
                                          xTq[eb * P:(eb + 1) * P, :])
                    for dg in range(DGN):
                        psq = [ps1.tile([P, QH], f32, tag="ps",
                                        name=f"psq_{dg}_{j}") for j in range(4)]
                        for eb in range(EB):
                            a4 = w1p.tile([P, 512], bf16, tag="w",
                                          name=f"a_{dg}_{eb}")
                            nc.sync.dma_start(a4[:], Ah[eb, dg])
                            for j in range(4):
                                nc.tensor.matmul(
                                    psq[j][:], a4[:, j * P:(j + 1) * P],
                                    xtq[:, eb, :],
                                    start=(eb == 0), stop=(eb == EB - 1))
                        for j in range(4):
                            dblk = dg * 4 + j
                            nc.scalar.activation(
                                tt[:, dblk, :], psq[j][:], Ident,
                                bias=u_sb[:, dblk:dblk + 1], scale=1.0)

                # --- EW: ew_r[ds] = colsum-ready cvw.T*V0 partials --------
                # (depends only on the gathered V, runs on gpsimd + spare
                #  DMA while the tensor engine is busy with T)
                ew_rs = []
                with (
                    tc.tile_pool(name="vew", bufs=6) as vewp,
                    tc.tile_pool(name="cvew", bufs=6) as cvewp,
                    tc.tile_pool(name="ewp", bufs=10) as ewp0,
                ):
                    for ds in range(NDS):
                        ew_acc = ewp0.tile([P, 512], f32, tag="ewa",
                                           name=f"ewa_{ds}")
                        for kb in range(KB):
                            vt = vewp.tile([P, 512], bf16, tag="v",
                                           name=f"vew_{ds}_{kb}")
                            nc.sync.dma_start(
                                vt[:], v_gth[ds % NDSH, ds // NDSH, kb])
                            cvt = cvewp.tile([P, 512], bf16, tag="cv",
                                             name=f"cvew_{ds}_{kb}")
                            nc.sync.dma_start(cvt[:], cvT[ds, :, kb, :])
                            if kb == 0:
                                nc.vector.tensor_mul(ew_acc[:], vt[:], cvt[:])
                            else:
                                ew = ewp0.tile([P, 512], f32, tag="ew",
                                               name=f"ew_{ds}_{kb}")
                                nc.vector.tensor_mul(ew[:], vt[:], cvt[:])
                                nc.vector.tensor_add(ew_acc[:], ew_acc[:],
                                                     ew[:])
                        # lives in the long-lived ptp pool: read by O phase
                        ew_r = ptp.tile([P, 512], f32r, tag=f"ewr{ds}",
                                        name=f"ewr_{ds}")
                        nc.vector.tensor_copy(ew_r[:], ew_acc[:])
                        ew_rs.append(ew_r)

                # ------- L: logits (bf16), softmax, P^T -------------------
                with (
                    tc.tile_pool(name="pp", bufs=2) as pp,
                    tc.tile_pool(name="sm", bufs=16) as smp,
                    tc.tile_pool(name="ps3", bufs=8, space="PSUM") as ps3,
                ):
                    lg = [[ps3.tile([P, NL], f32, tag="ps", name=f"lg_{qs}_{kh}")
                           for kh in range(KN)] for qs in range(QS)]
                    # qs-outer so lg[0] finishes early and its softmax +
                    # transposes overlap the remaining logits matmuls
                    for qs in range(QS):
                        for db in range(DB):
                            for kh in range(KN):
                                nc.tensor.matmul(
                                    lg[qs][kh][:],
                                    tt[:, db, qs * P:(qs + 1) * P],
                                    xte(db)[:, kh * NL:(kh + 1) * NL],
                                    start=(db == 0), stop=(db == DB - 1))
                    # P^T holds UNNORMALIZED exp; 1/z is applied as the
                    # activation scale on the final PSUM->SBUF copy, so the
                    # transposes start right after exp (no vector chain in
                    # the critical path).
                    rs = []
                    for qs in range(QS):
                        p_t = pp.tile([P, L], bf16, tag="p", name=f"p_{qs}")
                        zs = []
                        for kh in range(KN):
                            z = smp.tile([P, 1], f32, tag="sm",
                                         name=f"z_{qs}_{kh}")
                            nc.scalar.activation(
                                p_t[:, kh * NL:(kh + 1) * NL], lg[qs][kh][:],
                                Exp, scale=scale, accum_out=z[:])
                            zs.append(z)
                        for kb in range(KB):
                            pst = ps3.tile([P, P], bf16, tag="ps",
                                           name=f"pst_{qs}_{kb}")
                            nc.tensor.transpose(
                                pst[:], p_t[:, kb * P:(kb + 1) * P], ident[:])
                            nc.vector.tensor_copy(
                                pt_sb[:, kb, qs * P:(qs + 1) * P], pst[:])
                        zfull = zs[0]
                        for kh in range(1, KN):
                            z2 = smp.tile([P, 1], f32, tag="sm",
                                          name=f"zz_{qs}_{kh}")
                            nc.vector.tensor_add(z2[:], zfull[:], zs[kh][:])
                            zfull = z2
                        r = ptp.tile([P, 1], f32, tag=f"r{qs}",
                                     name=f"r_{qs}")
                        nc.vector.reciprocal(r[:], zfull[:])
                        rs.append(r)

                # ------- O: out = P^T.T @ V0 + ones*colsum(cvw.T*V0) ------
                with (
                    tc.tile_pool(name="vl", bufs=12) as vlp,
                    tc.tile_pool(name="ob", bufs=4) as obp,
                    tc.tile_pool(name="psO", bufs=8, space="PSUM") as psO,
                ):
                    for ds in range(NDS):
                        pso = [psO.tile([P, 512], f32, tag="po",
                                        name=f"pso_{ds}_{qs}")
                               for qs in range(QS)]
                        for kb in range(KB):
                            vt = vlp.tile([P, 512], bf16, tag="v",
                                          name=f"vl_{ds}_{kb}")
                            nc.sync.dma_start(
                                vt[:], v_gth[ds % NDSH, ds // NDSH, kb])
                            for qs in range(QS):
                                nc.tensor.matmul(
                                    pso[qs][:],
                                    pt_sb[:, kb, qs * P:(qs + 1) * P], vt[:],
                                    start=(kb == 0), stop=(kb == KB - 1))
                        # conv term: colsum(ew_r) broadcast to all 128 rows
                        ec_ps = psO.tile([P, 512], f32, tag="po",
                                         name=f"ec_{ds}")
                        nc.tensor.matmul(ec_ps[:], ones[:], ew_rs[ds][:],
                                         start=True, stop=True)
                        ecb = obp.tile([P, 512], f32, tag="ec",
                                       name=f"ecb_{ds}")
                        nc.vector.tensor_copy(ecb[:], ec_ps[:])
                        for qs in range(QS):
                            osb = obp.tile([P, 512], f32, tag="o",
                                           name=f"o_{ds}_{qs}")
                            nc.scalar.activation(osb[:], pso[qs][:], Ident,
                                                 scale=rs[qs][:])
                            nc.vector.tensor_add(osb[:], osb[:], ecb[:])
                            nc.sync.dma_start(
                                out[qs * P:(qs + 1) * P,
                                    ds * 512:(ds + 1) * 512], osb[:])
    nc.compile()
    return nc


# ----------------------------------------------------------------------
# Host side
# ----------------------------------------------------------------------

_CACHE = {}


def _get_nc(key, cfg):
    if key not in _CACHE:
        _CACHE[key] = build(cfg)
    return _CACHE[key]


def _bf16(a):
    import ml_dtypes
    return np.ascontiguousarray(a, dtype=ml_dtypes.bfloat16)


def _fp8(a):
    import ml_dtypes
    return np.ascontiguousarray(
        np.clip(np.asarray(a, np.float32), -240.0, 240.0),
        dtype=ml_dtypes.float8_e4m3)


def _prep_shared(cfg, wq, bq, wk, wv, cvw):
    EB, DGN, NDS, KB, DB = (cfg["EB"], cfg["DGN"], cfg["NDS"],
                            cfg["KB"], cfg["DB"])
    ECW, ECN = cfg["ECW"], cfg["ECN"]
    EBH = EB // 2
    wq = np.asarray(wq, np.float32)
    wk = np.asarray(wk, np.float32)
    A = wq.T @ wk                       # [e, d]
    u = np.asarray(bq, np.float32) @ wk  # [d]
    Ah = _bf16(A.reshape(EB, P, DGN, 512).transpose(0, 2, 1, 3))
    wvTh = _bf16(np.asarray(wv, np.float32).T
                 .reshape(ECN, ECW, P, NDS, 512).transpose(3, 0, 2, 1, 4))
    cvTh = _bf16(np.asarray(cvw, np.float32).T
                 .reshape(KB, P, NDS, 512).transpose(2, 1, 0, 3))
    uh = np.ascontiguousarray(u.reshape(DB, P).T, dtype=np.float32)
    return Ah, wvTh, cvTh, uh


def make_in_maps(cfg, x, wq, bq, wk, wv, cvw):
    QH, NDSH = cfg["QH"], cfg["NDSH"]
    B = x.shape[0]
    n_cores = B * (cfg["L"] // QH)
    Ah, wvTh, cvTh, uh = _prep_shared(cfg, wq, bq, wk, wv, cvw)
    wvT_halves = [np.ascontiguousarray(wvTh[:NDSH]),
                  np.ascontiguousarray(wvTh[NDSH:])]
    ones_h = np.ones((P, P), dtype=np.float32)
    in_maps = []
    for c in range(n_cores):
        b, ch = c // 2, c % 2
        xbT = np.asarray(x[b], np.float32).T
        in_maps.append(dict(
            xT=_bf16(xbT),
            xTq=_bf16(xbT[:, ch * QH:(ch + 1) * QH]),
            Ah=Ah, wvT=wvT_halves[ch], cvT=cvTh, uh=uh, onesd=ones_h,
        ))
    return in_maps, n_cores


def host_add_vec(bv, cvw, cvb):
    bv = np.asarray(bv, np.float32)
    cvw = np.asarray(cvw, np.float32)
    cvb = np.asarray(cvb, np.float32)
    return (bv * (1.0 + cvw.sum(axis=1)) + cvb).astype(np.float32)


def _gather(cfg, results, B, bv, cvw, cvb):
    QH, L, D = cfg["QH"], cfg["L"], cfg["D"]
    out = np.empty((B, L, D), dtype=np.float32)
    for c in range(2 * B):
        b, ch = c // 2, c % 2
        out[b, ch * QH:(ch + 1) * QH, :] = results[c]["out"]
    out += host_add_vec(bv, cvw, cvb)[None, None, :]
    return out


def kernel(x, wq, bq, wk, bk, wv, bv, ckw, ckb, cvw, cvb):
    """Full-input entry point. bk/ckw/ckb are mathematically dead (see top)."""
    from concourse.bass_utils import run_bass_kernel_spmd

    x = np.asarray(x, dtype=np.float32)
    cfg = _cfg(4096, 1024, 512)
    in_maps, n_cores = make_in_maps(cfg, x, wq, bq, wk, wv, cvw)
    nc = _get_nc(("full", 4096, 1024, 512), cfg)
    res = run_bass_kernel_spmd(nc, in_maps, core_ids=list(range(n_cores)))
    return _gather(cfg, res.results, x.shape[0], bv, cvw, cvb)


# revision 36
# speedup vs baseline: 1.0027x; 1.0027x over previous
"""Trainium2 Bass kernel for nn_FMA_15427522527280 (sparse_attention).

Math (B=4, L=1024, D=4096):
  Q = x@wq.T + bq ; K = x@wk.T + bk ; V = x@wv.T + bv
  out0 = softmax(Q K^T / sqrt(D)) @ V
  Level-1: softmax over a SINGLE key => s1 == 1.0 exactly, so
  out1 = V1 = depthwise_conv(V, cvw, cvb) broadcast over seq.
  out = out0 + out1

Exact simplifications:
  - logits = Q K^T = x (wq^T wk) x^T + 1_q (bq wk) x^T  (+ terms that are
    per-query constants over keys, which softmax drops).  A = wq^T wk is
    precomputed on the host => the K projection GEMM disappears, and the
    Q projection becomes T = x @ A + 1 (bq wk).
  - bv & cvb fold into a host-side per-feature constant:
      host_add[d] = bv[d]*(1 + sum_k cvw[d,k]) + cvb[d]
    (softmax rows sum to 1), device computes
      S@V0 + colsum_k(cvw[d,k]*V0[k,d])   with V0 = x@wv.T

Numerics: all GEMMs bf16 (measured rel-err ~3.9e-3 vs 2e-2 budget;
fp8/DoubleRow was tried and rejected: 2.1e-2 on the real data);
accumulation fp32 in PSUM; final out fp32.

Sharding: 8 cores = 4 batches x 2 query-halves.  The V projection is
split over the pair by output-feature half and exchanged with per-slice
HBM AllGathers (replica groups {2b, 2b+1}) that overlap the remaining
V compute; everything else is per-core.

Phases per core (xT resident in SBUF as bf16 throughout):
  V:  V0[k, d-half] = xT.T @ wvT(half)  -> DRAM, AllGather -> full V0
  T:  TT[d,q] = A^T @ xTq + u           -> SBUF resident (bf16)
  EW: ew_r[ds] = sum_kb cvw.T*V0        (vector engine, under T)
  L:  logits -> softmax (no max-sub; logits*scale ~ N(0,1)) -> P^T
  O:  out = P^T.T @ V0 + ones*ew_r
"""

import numpy as np

P = 128


def _cfg(D, L, QH):
    assert D % 512 == 0 and L % P == 0 and QH % P == 0
    EB = D // P
    cfg = dict(
        D=D, L=L, QH=QH,
        EB=EB,                 # input-feature blocks (contraction)
        DB=D // P,             # T feature blocks
        DGN=D // 512,          # 512-wide output groups for T
        QS=QH // P,            # query subtiles
        KB=L // P,             # key blocks
        NL=min(512, L),        # logits N tile
        NDS=D // 512,          # 512-wide d slices for V/out
        ECW=min(8, EB),        # wv chunk width (e-blocks per chunk)
    )
    cfg["KN"] = L // cfg["NL"]
    cfg["ECN"] = EB // cfg["ECW"]
    cfg["NDSH"] = cfg["NDS"] // 2   # V d-slices computed per core
    assert EB % cfg["ECW"] == 0
    assert cfg["KB"] <= 8, "V accumulators use one PSUM bank per key block"
    return cfg


def build(cfg):
    from concourse import bacc
    import concourse.mybir as mybir
    import concourse.tile as tile
    from concourse.masks import make_identity

    f32 = mybir.dt.float32
    f32r = mybir.dt.float32r
    bf16 = mybir.dt.bfloat16
    Ident = mybir.ActivationFunctionType.Identity
    Exp = mybir.ActivationFunctionType.Exp

    D, L, QH = cfg["D"], cfg["L"], cfg["QH"]
    EB, DB, DGN = cfg["EB"], cfg["DB"], cfg["DGN"]
    QS, KB, NL, KN = cfg["QS"], cfg["KB"], cfg["NL"], cfg["KN"]
    NDS, ECW, ECN, NDSH = cfg["NDS"], cfg["ECW"], cfg["ECN"], cfg["NDSH"]
    scale = 1.0 / float(np.sqrt(D))

    nc = bacc.Bacc("TRN2", target_bir_lowering=False)

    xT = nc.dram_tensor("xT", [D, L], bf16, kind="ExternalInput")
    xTq = nc.dram_tensor("xTq", [D, QH], bf16, kind="ExternalInput")
    Ah = nc.dram_tensor("Ah", [EB, DGN, P, 512], bf16, kind="ExternalInput")
    wvT = nc.dram_tensor("wvT", [NDSH, ECN, P, ECW, 512], bf16,
                         kind="ExternalInput")
    cvT = nc.dram_tensor("cvT", [NDS, P, KB, 512], bf16, kind="ExternalInput")
    uh = nc.dram_tensor("uh", [P, DB], f32, kind="ExternalInput")
    onesd = nc.dram_tensor("onesd", [P, P], f32r, kind="ExternalInput")
    out = nc.dram_tensor("out", [QH, D], f32, kind="ExternalOutput")

    v_loc = nc.dram_tensor("v_loc", [NDSH, KB, P, 512], bf16)
    # ds-major so each ds-slice can be gathered as soon as it is computed
    v_gth = nc.dram_tensor("v_gth", [NDSH, 2, KB, P, 512], bf16)
    rgroups = [[0, 1], [2, 3], [4, 5], [6, 7]]

    with tile.TileContext(nc) as tc:
        with tc.tile_pool(name="const", bufs=1) as constp:
            ones = constp.tile([P, P], f32r, tag="ones", name="ones")
            nc.sync.dma_start(ones[:], onesd[:])
            u_sb = constp.tile([P, DB], f32, tag="usb", name="u_sb")
            nc.sync.dma_start(u_sb[:], uh[:])
            ident = constp.tile([P, P], bf16, tag="ident", name="ident")
            make_identity(nc, ident)

            with (
                tc.tile_pool(name="xt", bufs=1) as xtp,
                tc.tile_pool(name="tt", bufs=1) as ttp,
                tc.tile_pool(name="ptp", bufs=1) as ptp,
            ):
                EBL = EB // 4
                xts = [xtp.tile([P, EBL, L], bf16, tag=f"xt{i}",
                                name=f"xt_{i}") for i in range(4)]
                for eb in range(EB):
                    nc.sync.dma_start(xts[eb // EBL][:, eb % EBL, :],
                                      xT[eb * P:(eb + 1) * P, :])

                def xte(eb):
                    return xts[eb // EBL][:, eb % EBL]

                xtq = xtp.tile([P, EB, QH], bf16, tag="xtq", name="xtq")
                tt = ttp.tile([P, DB, QH], bf16, tag="tt", name="tt")
                pt_sb = ptp.tile([P, KB, QH], bf16, tag="pt", name="pt_sb")

                # --- V: V0[k, d-half] = x @ wv^T(half) -> AllGather -------
                with (
                    tc.tile_pool(name="wv", bufs=3) as wvp,
                    tc.tile_pool(name="vcb", bufs=6) as vcb,
                    tc.tile_pool(name="psv", bufs=8, space="PSUM") as psvp,
                ):
                    for ds in range(NDSH):
                        psv = [psvp.tile([P, 512], f32, tag="ps",
                                         name=f"psv_{ds}_{kb}")
                               for kb in range(KB)]
                        for ec in range(ECN):
                            wc = wvp.tile([P, ECW, 512], bf16, tag="wv",
                                          name=f"wv_{ds}_{ec}")
                            nc.sync.dma_start(wc[:], wvT[ds, ec])
                            for j in range(ECW):
                                eb = ec * ECW + j
                                for kb in range(KB):
                                    nc.tensor.matmul(
                                        psv[kb][:],
                                        xte(eb)[:, kb * P:(kb + 1) * P],
                                        wc[:, j, :],
                                        start=(eb == 0), stop=(eb == EB - 1))
                        for kb in range(KB):
                            vsb = vcb.tile([P, 512], bf16, tag="v",
                                           name=f"v_{ds}_{kb}")
                            nc.vector.tensor_copy(vsb[:], psv[kb][:])
                            nc.sync.dma_start(v_loc[ds, kb], vsb[:])
                        # gather this slice while the next one computes
                        nc.gpsimd.collective_compute(
                            "AllGather", mybir.AluOpType.bypass,
                            replica_groups=rgroups,
                            ins=[v_loc[ds].opt()],
                            outs=[v_gth[ds].opt()])

                # --- T: TT[d,q] = A^T @ xq + u  (bf16) --------------------
                with (
                    tc.tile_pool(name="w1", bufs=10) as w1p,
                    tc.tile_pool(name="ps1", bufs=8, space="PSUM") as ps1,
                ):
                    # issued here so these DMAs queue behind the V-phase's
                    # xt/wv loads rather than ahead of them
                    for eb in range(EB):
                        nc.sync.dma_start(xtq[:, eb, :],
                                          xTq[eb * P:(eb + 1) * P, :])
                    for dg in range(DGN):
                        psq = [ps1.tile([P, QH], f32, tag="ps",
                                        name=f"psq_{dg}_{j}") for j in range(4)]
                        for eb in range(EB):
                            a4 = w1p.tile([P, 512], bf16, tag="w",
                                          name=f"a_{dg}_{eb}")
                            nc.sync.dma_start(a4[:], Ah[eb, dg])
                            for j in range(4):
                                nc.tensor.matmul(
                                    psq[j][:], a4[:, j * P:(j + 1) * P],
                                    xtq[:, eb, :],
                                    start=(eb == 0), stop=(eb == EB - 1))
                        for j in range(4):
                            dblk = dg * 4 + j
                            nc.scalar.activation(
                                tt[:, dblk, :], psq[j][:], Ident,
                                bias=u_sb[:, dblk:dblk + 1], scale=1.0)

                # --- EW: ew_r[ds] = colsum-ready cvw.T*V0 partials --------
                # (depends only on the gathered V, runs on gpsimd + spare
                #  DMA while the tensor engine is busy with T)
                ew_rs = []
                with (
                    tc.tile_pool(name="vew", bufs=6) as vewp,
                    tc.tile_pool(name="cvew", bufs=6) as cvewp,
                    tc.tile_pool(name="ewp", bufs=10) as ewp0,
                ):
                    for ds in range(NDS):
                        ew_acc = ewp0.tile([P, 512], f32, tag="ewa",
                                           name=f"ewa_{ds}")
                        for kb in range(KB):
                            vt = vewp.tile([P, 512], bf16, tag="v",
                                           name=f"vew_{ds}_{kb}")
                            nc.sync.dma_start(
                                vt[:], v_gth[ds % NDSH, ds // NDSH, kb])
                            cvt = cvewp.tile([P, 512], bf16, tag="cv",
                                             name=f"cvew_{ds}_{kb}")
                            nc.sync.dma_start(cvt[:], cvT[ds, :, kb, :])
                            if kb == 0:
                                nc.vector.tensor_mul(ew_acc[:], vt[:], cvt[:])
                            else:
                                ew = ewp0.tile([P, 512], f32, tag="ew",
                                               name=f"ew_{ds}_{kb}")
                                nc.vector.tensor_mul(ew[:], vt[:], cvt[:])
                                nc.vector.tensor_add(ew_acc[:], ew_acc[:],
                                                     ew[:])
                        # lives in the long-lived ptp pool: read by O phase
                        ew_r = ptp.tile([P, 512], f32r, tag=f"ewr{ds}",
                                        name=f"ewr_{ds}")
                        nc.vector.tensor_copy(ew_r[:], ew_acc[:])
                        ew_rs.append(ew_r)

                # ------- L: logits (bf16), softmax, P^T -------------------
                with (
                    tc.tile_pool(name="pp", bufs=2) as pp,
                    tc.tile_pool(name="sm", bufs=16) as smp,
                    tc.tile_pool(name="ps3", bufs=8, space="PSUM") as ps3,
                ):
                    lg = [[ps3.tile([P, NL], f32, tag="ps", name=f"lg_{qs}_{kh}")
                           for kh in range(KN)] for qs in range(QS)]
                    # qs-outer so lg[0] finishes early and its softmax +
                    # transposes overlap the remaining logits matmuls
                    for qs in range(QS):
                        for db in range(DB):
                            for kh in range(KN):
                                nc.tensor.matmul(
                                    lg[qs][kh][:],
                                    tt[:, db, qs * P:(qs + 1) * P],
                                    xte(db)[:, kh * NL:(kh + 1) * NL],
                                    start=(db == 0), stop=(db == DB - 1))
                    # P^T holds UNNORMALIZED exp; 1/z is applied as the
                    # activation scale on the final PSUM->SBUF copy, so the
                    # transposes start right after exp (no vector chain in
                    # the critical path).
                    rs = []
                    for qs in range(QS):
                        p_t = pp.tile([P, L], bf16, tag="p", name=f"p_{qs}")
                        zs = []
                        for kh in range(KN):
                            z = smp.tile([P, 1], f32, tag="sm",
                                         name=f"z_{qs}_{kh}")
                            nc.scalar.activation(
                                p_t[:, kh * NL:(kh + 1) * NL], lg[qs][kh][:],
                                Exp, scale=scale, accum_out=z[:])
                            zs.append(z)
                        for kb in range(KB):
                            pst = ps3.tile([P, P], bf16, tag="ps",
                                           name=f"pst_{qs}_{kb}")
                            nc.tensor.transpose(
                                pst[:], p_t[:, kb * P:(kb + 1) * P], ident[:])
                            nc.vector.tensor_copy(
                                pt_sb[:, kb, qs * P:(qs + 1) * P], pst[:])
                        zfull = zs[0]
                        for kh in range(1, KN):
                            z2 = smp.tile([P, 1], f32, tag="sm",
                                          name=f"zz_{qs}_{kh}")
                            nc.vector.tensor_add(z2[:], zfull[:], zs[kh][:])
                            zfull = z2
                        r = ptp.tile([P, 1], f32, tag=f"r{qs}",
                                     name=f"r_{qs}")
                        nc.vector.reciprocal(r[:], zfull[:])
                        rs.append(r)

                # ------- O: out = P^T.T @ V0 + ones*colsum(cvw.T*V0) ------
                with (
                    tc.tile_pool(name="vl", bufs=12) as vlp,
                    tc.tile_pool(name="ob", bufs=4) as obp,
                    tc.tile_pool(name="psO", bufs=8, space="PSUM") as psO,
                ):
                    for ds in range(NDS):
                        pso = [psO.tile([P, 512], f32, tag="po",
                                        name=f"pso_{ds}_{qs}")
                               for qs in range(QS)]
                        for kb in range(KB):
                            vt = vlp.tile([P, 512], bf16, tag="v",
                                          name=f"vl_{ds}_{kb}")
                            nc.sync.dma_start(
                                vt[:], v_gth[ds % NDSH, ds // NDSH, kb])
                            for qs in range(QS):
                                nc.tensor.matmul(
                                    pso[qs][:],
                                    pt_sb[:, kb, qs * P:(qs + 1) * P], vt[:],
                                    start=(kb == 0), stop=(kb == KB - 1))
                        # conv term: colsum(ew_r) broadcast to all 128 rows
                        ec_ps = psO.tile([P, 512], f32, tag="po",
                                         name=f"ec_{ds}")
                        nc.tensor.matmul(ec_ps[:], ones[:], ew_rs[ds][:],
                                         start=True, stop=True)
                        ecb = obp.tile([P, 512], f32, tag="ec",
                                       name=f"ecb_{ds}")
                        nc.vector.tensor_copy(ecb[:], ec_ps[:])
                        for qs in range(QS):
                            osb = obp.tile([P, 512], f32, tag="o",
                                           name=f"o_{ds}_{qs}")
                            nc.scalar.activation(osb[:], pso[qs][:], Ident,
                                                 scale=rs[qs][:])
                            nc.vector.tensor_add(osb[:], osb[:], ecb[:])
                            nc.sync.dma_start(
                                out[qs * P:(qs + 1) * P,
                                    ds * 512:(ds + 1) * 512], osb[:])
    nc.compile()
    return nc


# ----------------------------------------------------------------------
# Host side
# ----------------------------------------------------------------------

_CACHE = {}


def _get_nc(key, cfg):
    if key not in _CACHE:
        _CACHE[key] = build(cfg)
    return _CACHE[key]


def _bf16(a):
    import ml_dtypes
    return np.ascontiguousarray(a, dtype=ml_dtypes.bfloat16)


def _prep_shared(cfg, wq, bq, wk, wv, cvw):
    EB, DGN, NDS, KB, DB = (cfg["EB"], cfg["DGN"], cfg["NDS"],
                            cfg["KB"], cfg["DB"])
    ECW, ECN = cfg["ECW"], cfg["ECN"]
    wq = np.asarray(wq, np.float32)
    wk = np.asarray(wk, np.float32)
    A = wq.T @ wk                       # [e, d]
    u = np.asarray(bq, np.float32) @ wk  # [d]
    Ah = _bf16(A.reshape(EB, P, DGN, 512).transpose(0, 2, 1, 3))
    wvTh = _bf16(np.asarray(wv, np.float32).T
                 .reshape(ECN, ECW, P, NDS, 512).transpose(3, 0, 2, 1, 4))
    cvTh = _bf16(np.asarray(cvw, np.float32).T
                 .reshape(KB, P, NDS, 512).transpose(2, 1, 0, 3))
    uh = np.ascontiguousarray(u.reshape(DB, P).T, dtype=np.float32)
    return Ah, wvTh, cvTh, uh


def make_in_maps(cfg, x, wq, bq, wk, wv, cvw):
    QH, NDSH = cfg["QH"], cfg["NDSH"]
    B = x.shape[0]
    n_cores = B * (cfg["L"] // QH)
    Ah, wvTh, cvTh, uh = _prep_shared(cfg, wq, bq, wk, wv, cvw)
    wvT_halves = [np.ascontiguousarray(wvTh[:NDSH]),
                  np.ascontiguousarray(wvTh[NDSH:])]
    ones_h = np.ones((P, P), dtype=np.float32)
    in_maps = []
    for c in range(n_cores):
        b, ch = c // 2, c % 2
        xbT = np.asarray(x[b], np.float32).T
        in_maps.append(dict(
            xT=_bf16(xbT),
            xTq=_bf16(xbT[:, ch * QH:(ch + 1) * QH]),
            Ah=Ah, wvT=wvT_halves[ch], cvT=cvTh, uh=uh, onesd=ones_h,
        ))
    return in_maps, n_cores


def host_add_vec(bv, cvw, cvb):
    bv = np.asarray(bv, np.float32)
    cvw = np.asarray(cvw, np.float32)
    cvb = np.asarray(cvb, np.float32)
    return (bv * (1.0 + cvw.sum(axis=1)) + cvb).astype(np.float32)


def _gather(cfg, results, B, bv, cvw, cvb):
    QH, L, D = cfg["QH"], cfg["L"], cfg["D"]
    out = np.empty((B, L, D), dtype=np.float32)
    for c in range(2 * B):
        b, ch = c // 2, c % 2
        out[b, ch * QH:(ch + 1) * QH, :] = results[c]["out"]
    out += host_add_vec(bv, cvw, cvb)[None, None, :]
    return out


def kernel(x, wq, bq, wk, bk, wv, bv, ckw, ckb, cvw, cvb):
    """Full-input entry point. bk/ckw/ckb are mathematically dead (see top)."""
    from concourse.bass_utils import run_bass_kernel_spmd

    x = np.asarray(x, dtype=np.float32)
    cfg = _cfg(4096, 1024, 512)
    in_maps, n_cores = make_in_maps(cfg, x, wq, bq, wk, wv, cvw)
    nc = _get_nc(("full", 4096, 1024, 512), cfg)
    res = run_bass_kernel_spmd(nc, in_maps, core_ids=list(range(n_cores)))
    return _gather(cfg, res.results, x.shape[0], bv, cvw, cvb)


# revision 39
# speedup vs baseline: 1.0544x; 1.0516x over previous
"""Trainium2 Bass kernel for nn_FMA_15427522527280 (sparse_attention).

Math (B=4, L=1024, D=4096):
  Q = x@wq.T + bq ; K = x@wk.T + bk ; V = x@wv.T + bv
  out0 = softmax(Q K^T / sqrt(D)) @ V
  Level-1: softmax over a SINGLE key => s1 == 1.0 exactly, so
  out1 = V1 = depthwise_conv(V, cvw, cvb) broadcast over seq.
  out = out0 + out1

Exact simplifications:
  - logits = Q K^T = x (wq^T wk) x^T + 1_q (bq wk) x^T  (+ terms that are
    per-query constants over keys, which softmax drops).  A = wq^T wk is
    precomputed on the host => the K projection GEMM disappears, and the
    Q projection becomes T = x @ A + 1 (bq wk).
  - bv & cvb fold into a host-side per-feature constant:
      host_add[d] = bv[d]*(1 + sum_k cvw[d,k]) + cvb[d]
    (softmax rows sum to 1), device computes
      S@V0 + colsum_k(cvw[d,k]*V0[k,d])   with V0 = x@wv.T

Numerics: all GEMMs bf16 (measured rel-err ~3.9e-3 vs 2e-2 budget;
fp8/DoubleRow was tried and rejected: 2.1e-2 on the real data);
accumulation fp32 in PSUM; final out fp32.

Sharding: 8 cores = 4 batches x 2 query-halves.  The V projection is
split over the pair by output-feature half and exchanged with per-slice
HBM AllGathers (replica groups {2b, 2b+1}) that overlap the remaining
V compute; everything else is per-core.

Phases per core (xT resident in SBUF as bf16 throughout):
  V:  V0[k, d-half] = xT.T @ wvT(half)  -> DRAM, AllGather -> full V0
  T:  TT[d,q] = A^T @ xTq + u           -> SBUF resident (bf16)
  EW: ew_r[ds] = sum_kb cvw.T*V0        (vector engine, under T)
  L:  logits -> softmax (no max-sub; logits*scale ~ N(0,1)) -> P^T
  O:  out = P^T.T @ V0 + ones*ew_r
"""

import numpy as np

P = 128


def _cfg(D, L, QH):
    assert D % 512 == 0 and L % P == 0 and QH % P == 0
    EB = D // P
    cfg = dict(
        D=D, L=L, QH=QH,
        EB=EB,                 # input-feature blocks (contraction)
        DB=D // P,             # T feature blocks
        DGN=D // 512,          # 512-wide output groups for T
        QS=QH // P,            # query subtiles
        KB=L // P,             # key blocks
        NL=min(512, L),        # logits N tile
        NDS=D // 512,          # 512-wide d slices for V/out
        ECW=min(8, EB),        # wv chunk width (e-blocks per chunk)
    )
    cfg["KN"] = L // cfg["NL"]
    cfg["ECN"] = EB // cfg["ECW"]
    cfg["NDSH"] = cfg["NDS"] // 2   # V d-slices computed per core
    assert EB % cfg["ECW"] == 0
    assert cfg["KB"] <= 8, "V accumulators use one PSUM bank per key block"
    return cfg


def build(cfg):
    from concourse import bacc
    import concourse.mybir as mybir
    import concourse.tile as tile
    from concourse.masks import make_identity

    f32 = mybir.dt.float32
    f32r = mybir.dt.float32r
    bf16 = mybir.dt.bfloat16
    Ident = mybir.ActivationFunctionType.Identity
    Exp = mybir.ActivationFunctionType.Exp

    D, L, QH = cfg["D"], cfg["L"], cfg["QH"]
    EB, DB, DGN = cfg["EB"], cfg["DB"], cfg["DGN"]
    QS, KB, NL, KN = cfg["QS"], cfg["KB"], cfg["NL"], cfg["KN"]
    NDS, ECW, ECN, NDSH = cfg["NDS"], cfg["ECW"], cfg["ECN"], cfg["NDSH"]
    scale = 1.0 / float(np.sqrt(D))

    nc = bacc.Bacc("TRN2", target_bir_lowering=False)

    xT = nc.dram_tensor("xT", [P, EB, L], bf16, kind="ExternalInput")
    xTq = nc.dram_tensor("xTq", [P, EB, QH], bf16, kind="ExternalInput")
    Ah = nc.dram_tensor("Ah", [EB, DGN, P, 512], bf16, kind="ExternalInput")
    wvT = nc.dram_tensor("wvT", [NDSH, ECN, P, ECW, 512], bf16,
                         kind="ExternalInput")
    cvT = nc.dram_tensor("cvT", [NDS, P, KB, 512], bf16, kind="ExternalInput")
    uh = nc.dram_tensor("uh", [P, DB], f32, kind="ExternalInput")
    onesd = nc.dram_tensor("onesd", [P, P], f32r, kind="ExternalInput")
    out = nc.dram_tensor("out", [QH, D], f32, kind="ExternalOutput")

    v_loc = nc.dram_tensor("v_loc", [NDSH, P, KB, 512], bf16)
    # ds-major so each ds-slice can be gathered as soon as it is computed;
    # partition-major inside so one DMA loads a whole [P, KB, 512] slice
    v_gth = nc.dram_tensor("v_gth", [NDSH, 2, P, KB, 512], bf16)
    rgroups = [[0, 1], [2, 3], [4, 5], [6, 7]]

    with tile.TileContext(nc) as tc:
        with tc.tile_pool(name="const", bufs=1) as constp:
            ones = constp.tile([P, P], f32r, tag="ones", name="ones")
            nc.sync.dma_start(ones[:], onesd[:])
            u_sb = constp.tile([P, DB], f32, tag="usb", name="u_sb")
            nc.sync.dma_start(u_sb[:], uh[:])
            ident = constp.tile([P, P], bf16, tag="ident", name="ident")
            make_identity(nc, ident)

            with (
                tc.tile_pool(name="xt", bufs=1) as xtp,
                tc.tile_pool(name="tt", bufs=1) as ttp,
                tc.tile_pool(name="ptp", bufs=1) as ptp,
            ):
                EBL = EB // 4
                xts = [xtp.tile([P, EBL, L], bf16, tag=f"xt{i}",
                                name=f"xt_{i}") for i in range(4)]
                for i in range(4):
                    nc.sync.dma_start(xts[i][:],
                                      xT[:, i * EBL:(i + 1) * EBL, :])

                def xte(eb):
                    return xts[eb // EBL][:, eb % EBL]

                xtq = xtp.tile([P, EB, QH], bf16, tag="xtq", name="xtq")
                tt = ttp.tile([P, DB, QH], bf16, tag="tt", name="tt")
                pt_sb = ptp.tile([P, KB, QH], bf16, tag="pt", name="pt_sb")

                # --- V: V0[k, d-half] = x @ wv^T(half) -> AllGather -------
                with (
                    tc.tile_pool(name="wv", bufs=3) as wvp,
                    tc.tile_pool(name="vcb", bufs=6) as vcb,
                    tc.tile_pool(name="psv", bufs=8, space="PSUM") as psvp,
                ):
                    for ds in range(NDSH):
                        psv = [psvp.tile([P, 512], f32, tag="ps",
                                         name=f"psv_{ds}_{kb}")
                               for kb in range(KB)]
                        for ec in range(ECN):
                            wc = wvp.tile([P, ECW, 512], bf16, tag="wv",
                                          name=f"wv_{ds}_{ec}")
                            nc.sync.dma_start(wc[:], wvT[ds, ec])
                            for j in range(ECW):
                                eb = ec * ECW + j
                                for kb in range(KB):
                                    nc.tensor.matmul(
                                        psv[kb][:],
                                        xte(eb)[:, kb * P:(kb + 1) * P],
                                        wc[:, j, :],
                                        start=(eb == 0), stop=(eb == EB - 1))
                        for kb in range(KB):
                            vsb = vcb.tile([P, 512], bf16, tag="v",
                                           name=f"v_{ds}_{kb}")
                            nc.vector.tensor_copy(vsb[:], psv[kb][:])
                            nc.sync.dma_start(v_loc[ds, :, kb, :], vsb[:])
                        # gather this slice while the next one computes
                        nc.gpsimd.collective_compute(
                            "AllGather", mybir.AluOpType.bypass,
                            replica_groups=rgroups,
                            ins=[v_loc[ds].opt()],
                            outs=[v_gth[ds].opt()])

                # --- T: TT[d,q] = A^T @ xq + u  (bf16) --------------------
                with (
                    tc.tile_pool(name="w1", bufs=10) as w1p,
                    tc.tile_pool(name="ps1", bufs=8, space="PSUM") as ps1,
                ):
                    # issued here so this DMA queues behind the V-phase's
                    # xt/wv loads rather than ahead of them
                    nc.sync.dma_start(xtq[:], xTq[:])
                    for dg in range(DGN):
                        psq = [ps1.tile([P, QH], f32, tag="ps",
                                        name=f"psq_{dg}_{j}") for j in range(4)]
                        for eb in range(EB):
                            a4 = w1p.tile([P, 512], bf16, tag="w",
                                          name=f"a_{dg}_{eb}")
                            nc.sync.dma_start(a4[:], Ah[eb, dg])
                            for j in range(4):
                                nc.tensor.matmul(
                                    psq[j][:], a4[:, j * P:(j + 1) * P],
                                    xtq[:, eb, :],
                                    start=(eb == 0), stop=(eb == EB - 1))
                        for j in range(4):
                            dblk = dg * 4 + j
                            nc.scalar.activation(
                                tt[:, dblk, :], psq[j][:], Ident,
                                bias=u_sb[:, dblk:dblk + 1], scale=1.0)

                # --- EW: ew_r[ds] = colsum-ready cvw.T*V0 partials --------
                # (depends only on the gathered V, runs on gpsimd + spare
                #  DMA while the tensor engine is busy with T)
                ew_rs = []
                with (
                    tc.tile_pool(name="vew", bufs=2) as vewp,
                    tc.tile_pool(name="cvew", bufs=2) as cvewp,
                    tc.tile_pool(name="ewp", bufs=1) as ewp0,
                ):
                    for ds in range(NDS):
                        vt = vewp.tile([P, KB, 512], bf16, tag="v",
                                       name=f"vew_{ds}")
                        nc.sync.dma_start(vt[:], v_gth[ds % NDSH, ds // NDSH])
                        cvt = cvewp.tile([P, KB, 512], bf16, tag="cv",
                                         name=f"cvew_{ds}")
                        nc.sync.dma_start(cvt[:], cvT[ds])
                        t1 = ewp0.tile([P, KB, 512], bf16, tag="t1",
                                       name=f"ewt1_{ds}")
                        nc.vector.tensor_mul(t1[:], vt[:], cvt[:])
                        t2 = ewp0.tile([P, KB // 2, 512], bf16, tag="t2",
                                       name=f"ewt2_{ds}")
                        nc.vector.tensor_add(t2[:], t1[:, 0:KB // 2, :],
                                             t1[:, KB // 2:KB, :])
                        t3 = ewp0.tile([P, KB // 4, 512], bf16, tag="t3",
                                       name=f"ewt3_{ds}")
                        nc.vector.tensor_add(t3[:], t2[:, 0:KB // 4, :],
                                             t2[:, KB // 4:KB // 2, :])
                        # lives in the long-lived ptp pool: read by O phase
                        ew_r = ptp.tile([P, 512], f32r, tag=f"ewr{ds}",
                                        name=f"ewr_{ds}")
                        nc.vector.tensor_add(ew_r[:], t3[:, 0, :], t3[:, 1, :])
                        ew_rs.append(ew_r)

                # ------- L: logits (bf16), softmax, P^T -------------------
                with (
                    tc.tile_pool(name="pp", bufs=2) as pp,
                    tc.tile_pool(name="sm", bufs=16) as smp,
                    tc.tile_pool(name="ps3", bufs=8, space="PSUM") as ps3,
                ):
                    lg = [[ps3.tile([P, NL], f32, tag="ps", name=f"lg_{qs}_{kh}")
                           for kh in range(KN)] for qs in range(QS)]
                    # qs-outer so lg[0] finishes early and its softmax +
                    # transposes overlap the remaining logits matmuls
                    for qs in range(QS):
                        for db in range(DB):
                            for kh in range(KN):
                                nc.tensor.matmul(
                                    lg[qs][kh][:],
                                    tt[:, db, qs * P:(qs + 1) * P],
                                    xte(db)[:, kh * NL:(kh + 1) * NL],
                                    start=(db == 0), stop=(db == DB - 1))
                    # P^T holds UNNORMALIZED exp; 1/z is applied as the
                    # activation scale on the final PSUM->SBUF copy, so the
                    # transposes start right after exp (no vector chain in
                    # the critical path).
                    rs = []
                    for qs in range(QS):
                        p_t = pp.tile([P, L], bf16, tag="p", name=f"p_{qs}")
                        zs = []
                        for kh in range(KN):
                            z = smp.tile([P, 1], f32, tag="sm",
                                         name=f"z_{qs}_{kh}")
                            nc.scalar.activation(
                                p_t[:, kh * NL:(kh + 1) * NL], lg[qs][kh][:],
                                Exp, scale=scale, accum_out=z[:])
                            zs.append(z)
                        for kb in range(KB):
                            pst = ps3.tile([P, P], bf16, tag="ps",
                                           name=f"pst_{qs}_{kb}")
                            nc.tensor.transpose(
                                pst[:], p_t[:, kb * P:(kb + 1) * P], ident[:])
                            nc.vector.tensor_copy(
                                pt_sb[:, kb, qs * P:(qs + 1) * P], pst[:])
                        zfull = zs[0]
                        for kh in range(1, KN):
                            z2 = smp.tile([P, 1], f32, tag="sm",
                                          name=f"zz_{qs}_{kh}")
                            nc.vector.tensor_add(z2[:], zfull[:], zs[kh][:])
                            zfull = z2
                        r = ptp.tile([P, 1], f32, tag=f"r{qs}",
                                     name=f"r_{qs}")
                        nc.vector.reciprocal(r[:], zfull[:])
                        rs.append(r)

                # ------- O: out = P^T.T @ V0 + ones*colsum(cvw.T*V0) ------
                with (
                    tc.tile_pool(name="vl", bufs=3) as vlp,
                    tc.tile_pool(name="ob", bufs=4) as obp,
                    tc.tile_pool(name="psO", bufs=8, space="PSUM") as psO,
                ):
                    for ds in range(NDS):
                        pso = [psO.tile([P, 512], f32, tag="po",
                                        name=f"pso_{ds}_{qs}")
                               for qs in range(QS)]
                        vt = vlp.tile([P, KB, 512], bf16, tag="v",
                                      name=f"vl_{ds}")
                        nc.sync.dma_start(vt[:], v_gth[ds % NDSH, ds // NDSH])
                        for kb in range(KB):
                            for qs in range(QS):
                                nc.tensor.matmul(
                                    pso[qs][:],
                                    pt_sb[:, kb, qs * P:(qs + 1) * P],
                                    vt[:, kb, :],
                                    start=(kb == 0), stop=(kb == KB - 1))
                        # conv term: colsum(ew_r) broadcast to all 128 rows
                        ec_ps = psO.tile([P, 512], f32, tag="po",
                                         name=f"ec_{ds}")
                        nc.tensor.matmul(ec_ps[:], ones[:], ew_rs[ds][:],
                                         start=True, stop=True)
                        ecb = obp.tile([P, 512], f32, tag="ec",
                                       name=f"ecb_{ds}")
                        nc.vector.tensor_copy(ecb[:], ec_ps[:])
                        for qs in range(QS):
                            osb = obp.tile([P, 512], f32, tag="o",
                                           name=f"o_{ds}_{qs}")
                            nc.scalar.activation(osb[:], pso[qs][:], Ident,
                                                 scale=rs[qs][:])
                            nc.vector.tensor_add(osb[:], osb[:], ecb[:])
                            nc.sync.dma_start(
                                out[qs * P:(qs + 1) * P,
                                    ds * 512:(ds + 1) * 512], osb[:])
    nc.compile()
    return nc


# ----------------------------------------------------------------------
# Host side
# ----------------------------------------------------------------------

_CACHE = {}


def _get_nc(key, cfg):
    if key not in _CACHE:
        _CACHE[key] = build(cfg)
    return _CACHE[key]


def _bf16(a):
    import ml_dtypes
    return np.ascontiguousarray(a, dtype=ml_dtypes.bfloat16)


def _prep_shared(cfg, wq, bq, wk, wv, cvw):
    EB, DGN, NDS, KB, DB = (cfg["EB"], cfg["DGN"], cfg["NDS"],
                            cfg["KB"], cfg["DB"])
    ECW, ECN = cfg["ECW"], cfg["ECN"]
    wq = np.asarray(wq, np.float32)
    wk = np.asarray(wk, np.float32)
    A = wq.T @ wk                       # [e, d]
    u = np.asarray(bq, np.float32) @ wk  # [d]
    Ah = _bf16(A.reshape(EB, P, DGN, 512).transpose(0, 2, 1, 3))
    wvTh = _bf16(np.asarray(wv, np.float32).T
                 .reshape(ECN, ECW, P, NDS, 512).transpose(3, 0, 2, 1, 4))
    cvTh = _bf16(np.asarray(cvw, np.float32).T
                 .reshape(KB, P, NDS, 512).transpose(2, 1, 0, 3))
    uh = np.ascontiguousarray(u.reshape(DB, P).T, dtype=np.float32)
    return Ah, wvTh, cvTh, uh


def make_in_maps(cfg, x, wq, bq, wk, wv, cvw):
    QH, NDSH = cfg["QH"], cfg["NDSH"]
    B = x.shape[0]
    n_cores = B * (cfg["L"] // QH)
    Ah, wvTh, cvTh, uh = _prep_shared(cfg, wq, bq, wk, wv, cvw)
    wvT_halves = [np.ascontiguousarray(wvTh[:NDSH]),
                  np.ascontiguousarray(wvTh[NDSH:])]
    ones_h = np.ones((P, P), dtype=np.float32)
    EB, L = cfg["EB"], cfg["L"]
    in_maps = []
    for c in range(n_cores):
        b, ch = c // 2, c % 2
        # [P, EB, L] SBUF layout so the device loads x in 4 big DMAs
        xbT = np.asarray(x[b], np.float32).T.reshape(EB, P, L)
        xbT = xbT.transpose(1, 0, 2)
        in_maps.append(dict(
            xT=_bf16(xbT),
            xTq=_bf16(xbT[:, :, ch * QH:(ch + 1) * QH]),
            Ah=Ah, wvT=wvT_halves[ch], cvT=cvTh, uh=uh, onesd=ones_h,
        ))
    return in_maps, n_cores


def host_add_vec(bv, cvw, cvb):
    bv = np.asarray(bv, np.float32)
    cvw = np.asarray(cvw, np.float32)
    cvb = np.asarray(cvb, np.float32)
    return (bv * (1.0 + cvw.sum(axis=1)) + cvb).astype(np.float32)


def _gather(cfg, results, B, bv, cvw, cvb):
    QH, L, D = cfg["QH"], cfg["L"], cfg["D"]
    out = np.empty((B, L, D), dtype=np.float32)
    for c in range(2 * B):
        b, ch = c // 2, c % 2
        out[b, ch * QH:(ch + 1) * QH, :] = results[c]["out"]
    out += host_add_vec(bv, cvw, cvb)[None, None, :]
    return out


def kernel(x, wq, bq, wk, bk, wv, bv, ckw, ckb, cvw, cvb):
    """Full-input entry point. bk/ckw/ckb are mathematically dead (see top)."""
    from concourse.bass_utils import run_bass_kernel_spmd

    x = np.asarray(x, dtype=np.float32)
    cfg = _cfg(4096, 1024, 512)
    in_maps, n_cores = make_in_maps(cfg, x, wq, bq, wk, wv, cvw)
    nc = _get_nc(("full", 4096, 1024, 512), cfg)
    res = run_bass_kernel_spmd(nc, in_maps, core_ids=list(range(n_cores)))
    return _gather(cfg, res.results, x.shape[0], bv, cvw, cvb)


# revision 40
# speedup vs baseline: 1.0666x; 1.0116x over previous
"""Trainium2 Bass kernel for nn_FMA_15427522527280 (sparse_attention).

Math (B=4, L=1024, D=4096):
  Q = x@wq.T + bq ; K = x@wk.T + bk ; V = x@wv.T + bv
  out0 = softmax(Q K^T / sqrt(D)) @ V
  Level-1: softmax over a SINGLE key => s1 == 1.0 exactly, so
  out1 = V1 = depthwise_conv(V, cvw, cvb) broadcast over seq.
  out = out0 + out1

Exact simplifications:
  - logits = Q K^T = x (wq^T wk) x^T + 1_q (bq wk) x^T  (+ terms that are
    per-query constants over keys, which softmax drops).  A = wq^T wk is
    precomputed on the host => the K projection GEMM disappears, and the
    Q projection becomes T = x @ A + 1 (bq wk).
  - bv & cvb fold into a host-side per-feature constant:
      host_add[d] = bv[d]*(1 + sum_k cvw[d,k]) + cvb[d]
    (softmax rows sum to 1), device computes
      S@V0 + colsum_k(cvw[d,k]*V0[k,d])   with V0 = x@wv.T

Numerics: all GEMMs bf16 (measured rel-err ~3.9e-3 vs 2e-2 budget;
fp8/DoubleRow was tried and rejected: 2.1e-2 on the real data);
accumulation fp32 in PSUM; final out fp32.

Sharding: 8 cores = 4 batches x 2 query-halves.  The V projection is
split over the pair by output-feature half and exchanged with per-slice
HBM AllGathers (replica groups {2b, 2b+1}) that overlap the remaining
V compute; everything else is per-core.

Phases per core (xT resident in SBUF as bf16 throughout):
  V:  V0[k, d-half] = xT.T @ wvT(half)  -> DRAM, AllGather -> full V0
  T:  TT[d,q] = A^T @ xTq + u           -> SBUF resident (bf16)
  EW: ew_r[ds] = sum_kb cvw.T*V0        (vector engine, under T)
  L:  logits -> softmax (no max-sub; logits*scale ~ N(0,1)) -> P^T
  O:  out = P^T.T @ V0 + ones*ew_r
"""

import numpy as np

P = 128


def _cfg(D, L, QH):
    assert D % 512 == 0 and L % P == 0 and QH % P == 0
    EB = D // P
    cfg = dict(
        D=D, L=L, QH=QH,
        EB=EB,                 # input-feature blocks (contraction)
        DB=D // P,             # T feature blocks
        DGN=D // 512,          # 512-wide output groups for T
        QS=QH // P,            # query subtiles
        KB=L // P,             # key blocks
        NL=min(512, L),        # logits N tile
        NDS=D // 512,          # 512-wide d slices for V/out
        ECW=min(8, EB),        # wv chunk width (e-blocks per chunk)
    )
    cfg["KN"] = L // cfg["NL"]
    cfg["ECN"] = EB // cfg["ECW"]
    cfg["NDSH"] = cfg["NDS"] // 2   # V d-slices computed per core
    assert EB % cfg["ECW"] == 0
    assert cfg["KB"] <= 8, "V accumulators use one PSUM bank per key block"
    return cfg


def build(cfg):
    from concourse import bacc
    import concourse.mybir as mybir
    import concourse.tile as tile
    from concourse.masks import make_identity

    f32 = mybir.dt.float32
    f32r = mybir.dt.float32r
    bf16 = mybir.dt.bfloat16
    Ident = mybir.ActivationFunctionType.Identity
    Exp = mybir.ActivationFunctionType.Exp

    D, L, QH = cfg["D"], cfg["L"], cfg["QH"]
    EB, DB, DGN = cfg["EB"], cfg["DB"], cfg["DGN"]
    QS, KB, NL, KN = cfg["QS"], cfg["KB"], cfg["NL"], cfg["KN"]
    NDS, ECW, ECN, NDSH = cfg["NDS"], cfg["ECW"], cfg["ECN"], cfg["NDSH"]
    scale = 1.0 / float(np.sqrt(D))

    nc = bacc.Bacc("TRN2", target_bir_lowering=False)

    xT = nc.dram_tensor("xT", [P, EB, L], bf16, kind="ExternalInput")
    xTq = nc.dram_tensor("xTq", [P, EB, QH], bf16, kind="ExternalInput")
    Ah = nc.dram_tensor("Ah", [EB, DGN, P, 512], bf16, kind="ExternalInput")
    wvT = nc.dram_tensor("wvT", [NDSH, ECN, P, ECW, 512], bf16,
                         kind="ExternalInput")
    cvT = nc.dram_tensor("cvT", [NDS, P, KB, 512], bf16, kind="ExternalInput")
    uh = nc.dram_tensor("uh", [P, DB], f32, kind="ExternalInput")
    onesd = nc.dram_tensor("onesd", [P, P], f32r, kind="ExternalInput")
    out = nc.dram_tensor("out", [QH, D], f32, kind="ExternalOutput")

    v_loc = nc.dram_tensor("v_loc", [NDSH, P, KB, 512], bf16)
    # ds-major so each ds-slice can be gathered as soon as it is computed;
    # partition-major inside so one DMA loads a whole [P, KB, 512] slice
    v_gth = nc.dram_tensor("v_gth", [NDSH, 2, P, KB, 512], bf16)
    rgroups = [[0, 1], [2, 3], [4, 5], [6, 7]]

    with tile.TileContext(nc) as tc:
        with tc.tile_pool(name="const", bufs=1) as constp:
            ones = constp.tile([P, P], f32r, tag="ones", name="ones")
            nc.sync.dma_start(ones[:], onesd[:])
            u_sb = constp.tile([P, DB], f32, tag="usb", name="u_sb")
            nc.sync.dma_start(u_sb[:], uh[:])
            ident = constp.tile([P, P], bf16, tag="ident", name="ident")
            make_identity(nc, ident)

            with (
                tc.tile_pool(name="xt", bufs=1) as xtp,
                tc.tile_pool(name="tt", bufs=1) as ttp,
                tc.tile_pool(name="ptp", bufs=1) as ptp,
            ):
                EBL = EB // 4
                xts = [xtp.tile([P, EBL, L], bf16, tag=f"xt{i}",
                                name=f"xt_{i}") for i in range(4)]
                for i in range(4):
                    nc.sync.dma_start(xts[i][:],
                                      xT[:, i * EBL:(i + 1) * EBL, :])

                def xte(eb):
                    return xts[eb // EBL][:, eb % EBL]

                xtq = xtp.tile([P, EB, QH], bf16, tag="xtq", name="xtq")
                tt = ttp.tile([P, DB, QH], bf16, tag="tt", name="tt")
                pt_sb = ptp.tile([P, KB, QH], bf16, tag="pt", name="pt_sb")

                # --- V: V0[k, d-half] = x @ wv^T(half) -> AllGather -------
                with (
                    tc.tile_pool(name="wv", bufs=3) as wvp,
                    tc.tile_pool(name="vcb", bufs=6) as vcb,
                    tc.tile_pool(name="psv", bufs=8, space="PSUM") as psvp,
                ):
                    for ds in range(NDSH):
                        psv = [psvp.tile([P, 512], f32, tag="ps",
                                         name=f"psv_{ds}_{kb}")
                               for kb in range(KB)]
                        for ec in range(ECN):
                            wc = wvp.tile([P, ECW, 512], bf16, tag="wv",
                                          name=f"wv_{ds}_{ec}")
                            nc.sync.dma_start(wc[:], wvT[ds, ec])
                            for j in range(ECW):
                                eb = ec * ECW + j
                                for kb in range(KB):
                                    nc.tensor.matmul(
                                        psv[kb][:],
                                        xte(eb)[:, kb * P:(kb + 1) * P],
                                        wc[:, j, :],
                                        start=(eb == 0), stop=(eb == EB - 1))
                        if ds == 0:
                            nc.sync.dma_start(xtq[:], xTq[:])
                        for kb in range(KB):
                            vsb = vcb.tile([P, 512], bf16, tag="v",
                                           name=f"v_{ds}_{kb}")
                            nc.vector.tensor_copy(vsb[:], psv[kb][:])
                            nc.sync.dma_start(v_loc[ds, :, kb, :], vsb[:])
                        # gather this slice while the next one computes
                        nc.gpsimd.collective_compute(
                            "AllGather", mybir.AluOpType.bypass,
                            replica_groups=rgroups,
                            ins=[v_loc[ds].opt()],
                            outs=[v_gth[ds].opt()])

                # --- T: TT[d,q] = A^T @ xq + u  (bf16) --------------------
                with (
                    tc.tile_pool(name="w1", bufs=10) as w1p,
                    tc.tile_pool(name="ps1", bufs=8, space="PSUM") as ps1,
                ):
                    for dg in range(DGN):
                        psq = [ps1.tile([P, QH], f32, tag="ps",
                                        name=f"psq_{dg}_{j}") for j in range(4)]
                        for eb in range(EB):
                            a4 = w1p.tile([P, 512], bf16, tag="w",
                                          name=f"a_{dg}_{eb}")
                            nc.sync.dma_start(a4[:], Ah[eb, dg])
                            for j in range(4):
                                nc.tensor.matmul(
                                    psq[j][:], a4[:, j * P:(j + 1) * P],
                                    xtq[:, eb, :],
                                    start=(eb == 0), stop=(eb == EB - 1))
                        for j in range(4):
                            dblk = dg * 4 + j
                            nc.scalar.activation(
                                tt[:, dblk, :], psq[j][:], Ident,
                                bias=u_sb[:, dblk:dblk + 1], scale=1.0)

                # --- EW: ew_r[ds] = colsum-ready cvw.T*V0 partials --------
                # (depends only on the gathered V, runs on gpsimd + spare
                #  DMA while the tensor engine is busy with T)
                ew_rs = []
                with (
                    tc.tile_pool(name="vew", bufs=2) as vewp,
                    tc.tile_pool(name="cvew", bufs=2) as cvewp,
                    tc.tile_pool(name="ewp", bufs=1) as ewp0,
                ):
                    for ds in range(NDS):
                        vt = vewp.tile([P, KB, 512], bf16, tag="v",
                                       name=f"vew_{ds}")
                        nc.sync.dma_start(vt[:], v_gth[ds % NDSH, ds // NDSH])
                        cvt = cvewp.tile([P, KB, 512], bf16, tag="cv",
                                         name=f"cvew_{ds}")
                        nc.sync.dma_start(cvt[:], cvT[ds])
                        t1 = ewp0.tile([P, KB, 512], bf16, tag="t1",
                                       name=f"ewt1_{ds}")
                        nc.vector.tensor_mul(t1[:], vt[:], cvt[:])
                        t2 = ewp0.tile([P, KB // 2, 512], bf16, tag="t2",
                                       name=f"ewt2_{ds}")
                        nc.vector.tensor_add(t2[:], t1[:, 0:KB // 2, :],
                                             t1[:, KB // 2:KB, :])
                        t3 = ewp0.tile([P, KB // 4, 512], bf16, tag="t3",
                                       name=f"ewt3_{ds}")
                        nc.vector.tensor_add(t3[:], t2[:, 0:KB // 4, :],
                                             t2[:, KB // 4:KB // 2, :])
                        # lives in the long-lived ptp pool: read by O phase
                        ew_r = ptp.tile([P, 512], f32r, tag=f"ewr{ds}",
                                        name=f"ewr_{ds}")
                        nc.vector.tensor_add(ew_r[:], t3[:, 0, :], t3[:, 1, :])
                        ew_rs.append(ew_r)

                # ------- L: logits (bf16), softmax, P^T -------------------
                with (
                    tc.tile_pool(name="pp", bufs=2) as pp,
                    tc.tile_pool(name="sm", bufs=16) as smp,
                    tc.tile_pool(name="ps3", bufs=8, space="PSUM") as ps3,
                ):
                    lg = [[ps3.tile([P, NL], f32, tag="ps", name=f"lg_{qs}_{kh}")
                           for kh in range(KN)] for qs in range(QS)]
                    # qs-outer so lg[0] finishes early and its softmax +
                    # transposes overlap the remaining logits matmuls
                    for qs in range(QS):
                        for db in range(DB):
                            for kh in range(KN):
                                nc.tensor.matmul(
                                    lg[qs][kh][:],
                                    tt[:, db, qs * P:(qs + 1) * P],
                                    xte(db)[:, kh * NL:(kh + 1) * NL],
                                    start=(db == 0), stop=(db == DB - 1))
                    # P^T holds UNNORMALIZED exp; 1/z is applied as the
                    # activation scale on the final PSUM->SBUF copy, so the
                    # transposes start right after exp (no vector chain in
                    # the critical path).
                    rs = []
                    for qs in range(QS):
                        p_t = pp.tile([P, L], bf16, tag="p", name=f"p_{qs}")
                        zs = []
                        for kh in range(KN):
                            z = smp.tile([P, 1], f32, tag="sm",
                                         name=f"z_{qs}_{kh}")
                            nc.scalar.activation(
                                p_t[:, kh * NL:(kh + 1) * NL], lg[qs][kh][:],
                                Exp, scale=scale, accum_out=z[:])
                            zs.append(z)
                        for kb in range(KB):
                            pst = ps3.tile([P, P], bf16, tag="ps",
                                           name=f"pst_{qs}_{kb}")
                            nc.tensor.transpose(
                                pst[:], p_t[:, kb * P:(kb + 1) * P], ident[:])
                            nc.vector.tensor_copy(
                                pt_sb[:, kb, qs * P:(qs + 1) * P], pst[:])
                        zfull = zs[0]
                        for kh in range(1, KN):
                            z2 = smp.tile([P, 1], f32, tag="sm",
                                          name=f"zz_{qs}_{kh}")
                            nc.vector.tensor_add(z2[:], zfull[:], zs[kh][:])
                            zfull = z2
                        r = ptp.tile([P, 1], f32, tag=f"r{qs}",
                                     name=f"r_{qs}")
                        nc.vector.reciprocal(r[:], zfull[:])
                        rs.append(r)

                # ------- O: out = P^T.T @ V0 + ones*colsum(cvw.T*V0) ------
                with (
                    tc.tile_pool(name="vl", bufs=3) as vlp,
                    tc.tile_pool(name="ob", bufs=4) as obp,
                    tc.tile_pool(name="psO", bufs=8, space="PSUM") as psO,
                ):
                    for ds in range(NDS):
                        pso = [psO.tile([P, 512], f32, tag="po",
                                        name=f"pso_{ds}_{qs}")
                               for qs in range(QS)]
                        vt = vlp.tile([P, KB, 512], bf16, tag="v",
                                      name=f"vl_{ds}")
                        nc.sync.dma_start(vt[:], v_gth[ds % NDSH, ds // NDSH])
                        for kb in range(KB):
                            for qs in range(QS):
                                nc.tensor.matmul(
                                    pso[qs][:],
                                    pt_sb[:, kb, qs * P:(qs + 1) * P],
                                    vt[:, kb, :],
                                    start=(kb == 0), stop=(kb == KB - 1))
                        # conv term: colsum(ew_r) broadcast to all 128 rows
                        ec_ps = psO.tile([P, 512], f32, tag="po",
                                         name=f"ec_{ds}")
                        nc.tensor.matmul(ec_ps[:], ones[:], ew_rs[ds][:],
                                         start=True, stop=True)
                        ecb = obp.tile([P, 512], f32, tag="ec",
                                       name=f"ecb_{ds}")
                        nc.vector.tensor_copy(ecb[:], ec_ps[:])
                        for qs in range(QS):
                            osb = obp.tile([P, 512], f32, tag="o",
                                           name=f"o_{ds}_{qs}")
                            nc.scalar.activation(osb[:], pso[qs][:], Ident,
                                                 scale=rs[qs][:])
                            nc.vector.tensor_add(osb[:], osb[:], ecb[:])
                            nc.sync.dma_start(
                                out[qs * P:(qs + 1) * P,
                                    ds * 512:(ds + 1) * 512], osb[:])
    nc.compile()
    return nc


# ----------------------------------------------------------------------
# Host side
# ----------------------------------------------------------------------

_CACHE = {}


def _get_nc(key, cfg):
    if key not in _CACHE:
        _CACHE[key] = build(cfg)
    return _CACHE[key]


def _bf16(a):
    import ml_dtypes
    return np.ascontiguousarray(a, dtype=ml_dtypes.bfloat16)


def _prep_shared(cfg, wq, bq, wk, wv, cvw):
    EB, DGN, NDS, KB, DB = (cfg["EB"], cfg["DGN"], cfg["NDS"],
                            cfg["KB"], cfg["DB"])
    ECW, ECN = cfg["ECW"], cfg["ECN"]
    wq = np.asarray(wq, np.float32)
    wk = np.asarray(wk, np.float32)
    A = wq.T @ wk                       # [e, d]
    u = np.asarray(bq, np.float32) @ wk  # [d]
    Ah = _bf16(A.reshape(EB, P, DGN, 512).transpose(0, 2, 1, 3))
    wvTh = _bf16(np.asarray(wv, np.float32).T
                 .reshape(ECN, ECW, P, NDS, 512).transpose(3, 0, 2, 1, 4))
    cvTh = _bf16(np.asarray(cvw, np.float32).T
                 .reshape(KB, P, NDS, 512).transpose(2, 1, 0, 3))
    uh = np.ascontiguousarray(u.reshape(DB, P).T, dtype=np.float32)
    return Ah, wvTh, cvTh, uh


def make_in_maps(cfg, x, wq, bq, wk, wv, cvw):
    QH, NDSH = cfg["QH"], cfg["NDSH"]
    B = x.shape[0]
    n_cores = B * (cfg["L"] // QH)
    Ah, wvTh, cvTh, uh = _prep_shared(cfg, wq, bq, wk, wv, cvw)
    wvT_halves = [np.ascontiguousarray(wvTh[:NDSH]),
                  np.ascontiguousarray(wvTh[NDSH:])]
    ones_h = np.ones((P, P), dtype=np.float32)
    EB, L = cfg["EB"], cfg["L"]
    in_maps = []
    for c in range(n_cores):
        b, ch = c // 2, c % 2
        # [P, EB, L] SBUF layout so the device loads x in 4 big DMAs
        xbT = np.asarray(x[b], np.float32).T.reshape(EB, P, L)
        xbT = xbT.transpose(1, 0, 2)
        in_maps.append(dict(
            xT=_bf16(xbT),
            xTq=_bf16(xbT[:, :, ch * QH:(ch + 1) * QH]),
            Ah=Ah, wvT=wvT_halves[ch], cvT=cvTh, uh=uh, onesd=ones_h,
        ))
    return in_maps, n_cores


def host_add_vec(bv, cvw, cvb):
    bv = np.asarray(bv, np.float32)
    cvw = np.asarray(cvw, np.float32)
    cvb = np.asarray(cvb, np.float32)
    return (bv * (1.0 + cvw.sum(axis=1)) + cvb).astype(np.float32)


def _gather(cfg, results, B, bv, cvw, cvb):
    QH, L, D = cfg["QH"], cfg["L"], cfg["D"]
    out = np.empty((B, L, D), dtype=np.float32)
    for c in range(2 * B):
        b, ch = c // 2, c % 2
        out[b, ch * QH:(ch + 1) * QH, :] = results[c]["out"]
    out += host_add_vec(bv, cvw, cvb)[None, None, :]
    return out


def kernel(x, wq, bq, wk, bk, wv, bv, ckw, ckb, cvw, cvb):
    """Full-input entry point. bk/ckw/ckb are mathematically dead (see top)."""
    from concourse.bass_utils import run_bass_kernel_spmd

    x = np.asarray(x, dtype=np.float32)
    cfg = _cfg(4096, 1024, 512)
    in_maps, n_cores = make_in_maps(cfg, x, wq, bq, wk, wv, cvw)
    nc = _get_nc(("full", 4096, 1024, 512), cfg)
    res = run_bass_kernel_spmd(nc, in_maps, core_ids=list(range(n_cores)))
    return _gather(cfg, res.results, x.shape[0], bv, cvw, cvb)


# revision 42
# speedup vs baseline: 1.0797x; 1.0123x over previous
"""Trainium2 Bass kernel for nn_FMA_15427522527280 (sparse_attention).

Math (B=4, L=1024, D=4096):
  Q = x@wq.T + bq ; K = x@wk.T + bk ; V = x@wv.T + bv
  out0 = softmax(Q K^T / sqrt(D)) @ V
  Level-1: softmax over a SINGLE key => s1 == 1.0 exactly, so
  out1 = V1 = depthwise_conv(V, cvw, cvb) broadcast over seq.
  out = out0 + out1

Exact simplifications:
  - logits = Q K^T = x (wq^T wk) x^T + 1_q (bq wk) x^T  (+ terms that are
    per-query constants over keys, which softmax drops).  A = wq^T wk is
    precomputed on the host => the K projection GEMM disappears, and the
    Q projection becomes T = x @ A + 1 (bq wk).
  - bv & cvb fold into a host-side per-feature constant:
      host_add[d] = bv[d]*(1 + sum_k cvw[d,k]) + cvb[d]
    (softmax rows sum to 1), device computes
      S@V0 + colsum_k(cvw[d,k]*V0[k,d])   with V0 = x@wv.T

Numerics: all GEMMs bf16 (measured rel-err ~3.9e-3 vs 2e-2 budget;
fp8/DoubleRow was tried and rejected: 2.1e-2 on the real data);
accumulation fp32 in PSUM; final out fp32.

Sharding: 8 cores = 4 batches x 2 query-halves.  The V projection is
split over the pair by output-feature half and exchanged with per-slice
HBM AllGathers (replica groups {2b, 2b+1}) that overlap the remaining
V compute; everything else is per-core.

Phases per core (xT resident in SBUF as bf16 throughout):
  V:  V0[k, d-half] = xT.T @ wvT(half)  -> DRAM, AllGather -> full V0
  T:  TT[d,q] = A^T @ xTq + u           -> SBUF resident (bf16)
  EW: ew_r[ds] = sum_kb cvw.T*V0        (vector engine, under T)
  L:  logits -> softmax (no max-sub; logits*scale ~ N(0,1)) -> P^T
  O:  out = P^T.T @ V0 + ones*ew_r
"""

import numpy as np

P = 128


def _cfg(D, L, QH):
    assert D % 512 == 0 and L % P == 0 and QH % P == 0
    EB = D // P
    cfg = dict(
        D=D, L=L, QH=QH,
        EB=EB,                 # input-feature blocks (contraction)
        DB=D // P,             # T feature blocks
        DGN=D // 512,          # 512-wide output groups for T
        QS=QH // P,            # query subtiles
        KB=L // P,             # key blocks
        NL=min(512, L),        # logits N tile
        NDS=D // 512,          # 512-wide d slices for V/out
        ECW=min(8, EB),        # wv chunk width (e-blocks per chunk)
    )
    cfg["KN"] = L // cfg["NL"]
    cfg["ECN"] = EB // cfg["ECW"]
    cfg["NDSH"] = cfg["NDS"] // 2   # V d-slices computed per core
    assert EB % cfg["ECW"] == 0
    assert cfg["KB"] <= 8, "V accumulators use one PSUM bank per key block"
    return cfg


def build(cfg):
    from concourse import bacc
    import concourse.mybir as mybir
    import concourse.tile as tile
    from concourse.masks import make_identity

    f32 = mybir.dt.float32
    f32r = mybir.dt.float32r
    bf16 = mybir.dt.bfloat16
    Ident = mybir.ActivationFunctionType.Identity
    Exp = mybir.ActivationFunctionType.Exp

    D, L, QH = cfg["D"], cfg["L"], cfg["QH"]
    EB, DB, DGN = cfg["EB"], cfg["DB"], cfg["DGN"]
    QS, KB, NL, KN = cfg["QS"], cfg["KB"], cfg["NL"], cfg["KN"]
    NDS, ECW, ECN, NDSH = cfg["NDS"], cfg["ECW"], cfg["ECN"], cfg["NDSH"]
    scale = 1.0 / float(np.sqrt(D))

    nc = bacc.Bacc("TRN2", target_bir_lowering=False)

    xT = nc.dram_tensor("xT", [P, EB, L], bf16, kind="ExternalInput")
    xTq = nc.dram_tensor("xTq", [P, EB, QH], bf16, kind="ExternalInput")
    Ah = nc.dram_tensor("Ah", [EB, DGN, P, 512], bf16, kind="ExternalInput")
    wvT = nc.dram_tensor("wvT", [NDSH, ECN, P, ECW, 512], bf16,
                         kind="ExternalInput")
    cvT = nc.dram_tensor("cvT", [NDS, P, KB, 512], bf16, kind="ExternalInput")
    uh = nc.dram_tensor("uh", [P, DB], f32, kind="ExternalInput")
    onesd = nc.dram_tensor("onesd", [P, P], f32r, kind="ExternalInput")
    out = nc.dram_tensor("out", [QH, D], f32, kind="ExternalOutput")

    v_loc = nc.dram_tensor("v_loc", [NDSH, P, KB, 512], bf16)
    # ds-major so each ds-slice can be gathered as soon as it is computed;
    # partition-major inside so one DMA loads a whole [P, KB, 512] slice
    v_gth = nc.dram_tensor("v_gth", [NDSH, 2, P, KB, 512], bf16)
    rgroups = [[0, 1], [2, 3], [4, 5], [6, 7]]

    with tile.TileContext(nc) as tc:
        with tc.tile_pool(name="const", bufs=1) as constp:
            ones = constp.tile([P, P], f32r, tag="ones", name="ones")
            nc.sync.dma_start(ones[:], onesd[:])
            u_sb = constp.tile([P, DB], f32, tag="usb", name="u_sb")
            nc.sync.dma_start(u_sb[:], uh[:])
            ident = constp.tile([P, P], bf16, tag="ident", name="ident")
            make_identity(nc, ident)

            with (
                tc.tile_pool(name="xt", bufs=1) as xtp,
                tc.tile_pool(name="tt", bufs=1) as ttp,
                tc.tile_pool(name="ptp", bufs=1) as ptp,
                tc.tile_pool(name="w1", bufs=10) as w1p,
            ):
                a4_pre = []
                EBL = EB // 4
                xts = [xtp.tile([P, EBL, L], bf16, tag=f"xt{i}",
                                name=f"xt_{i}") for i in range(4)]
                nc.sync.dma_start(xts[0][:], xT[:, 0:EBL, :])

                def xte(eb):
                    return xts[eb // EBL][:, eb % EBL]

                xtq = xtp.tile([P, EB, QH], bf16, tag="xtq", name="xtq")
                tt = ttp.tile([P, DB, QH], bf16, tag="tt", name="tt")
                pt_sb = ptp.tile([P, KB, QH], bf16, tag="pt", name="pt_sb")

                # --- V: V0[k, d-half] = x @ wv^T(half) -> AllGather -------
                with (
                    tc.tile_pool(name="wv", bufs=3) as wvp,
                    tc.tile_pool(name="vcb", bufs=6) as vcb,
                    tc.tile_pool(name="psv", bufs=8, space="PSUM") as psvp,
                ):
                    for ds in range(NDSH):
                        psv = [psvp.tile([P, 512], f32, tag="ps",
                                         name=f"psv_{ds}_{kb}")
                               for kb in range(KB)]
                        for ec in range(ECN):
                            if ds == 0 and ec > 0:
                                nc.sync.dma_start(
                                    xts[ec][:],
                                    xT[:, ec * EBL:(ec + 1) * EBL, :])
                            if ds == 1 and ec == 0:
                                for pe in range(10):
                                    t = w1p.tile([P, 512], bf16, tag="w",
                                                 name=f"a_0_{pe}")
                                    nc.sync.dma_start(t[:], Ah[pe, 0])
                                    a4_pre.append(t)
                            wc = wvp.tile([P, ECW, 512], bf16, tag="wv",
                                          name=f"wv_{ds}_{ec}")
                            nc.sync.dma_start(wc[:], wvT[ds, ec])
                            for j in range(ECW):
                                eb = ec * ECW + j
                                for kb in range(KB):
                                    nc.tensor.matmul(
                                        psv[kb][:],
                                        xte(eb)[:, kb * P:(kb + 1) * P],
                                        wc[:, j, :],
                                        start=(eb == 0), stop=(eb == EB - 1))
                        if ds == 0:
                            nc.sync.dma_start(xtq[:], xTq[:])
                        for kb in range(KB):
                            vsb = vcb.tile([P, 512], bf16, tag="v",
                                           name=f"v_{ds}_{kb}")
                            nc.vector.tensor_copy(vsb[:], psv[kb][:])
                            nc.sync.dma_start(v_loc[ds, :, kb, :], vsb[:])
                        # gather this slice while the next one computes
                        nc.gpsimd.collective_compute(
                            "AllGather", mybir.AluOpType.bypass,
                            replica_groups=rgroups,
                            ins=[v_loc[ds].opt()],
                            outs=[v_gth[ds].opt()])

                # --- T: TT[d,q] = A^T @ xq + u  (bf16) --------------------
                with (
                    tc.tile_pool(name="ps1", bufs=8, space="PSUM") as ps1,
                ):
                    for dg in range(DGN):
                        psq = [ps1.tile([P, QH], f32, tag="ps",
                                        name=f"psq_{dg}_{j}") for j in range(4)]
                        for eb in range(EB):
                            if dg == 0 and eb < 10:
                                a4 = a4_pre[eb]
                            else:
                                a4 = w1p.tile([P, 512], bf16, tag="w",
                                              name=f"a_{dg}_{eb}")
                                nc.sync.dma_start(a4[:], Ah[eb, dg])
                            for j in range(4):
                                nc.tensor.matmul(
                                    psq[j][:], a4[:, j * P:(j + 1) * P],
                                    xtq[:, eb, :],
                                    start=(eb == 0), stop=(eb == EB - 1))
                        for j in range(4):
                            dblk = dg * 4 + j
                            nc.scalar.activation(
                                tt[:, dblk, :], psq[j][:], Ident,
                                bias=u_sb[:, dblk:dblk + 1], scale=1.0)

                # --- EW: ew_r[ds] = colsum-ready cvw.T*V0 partials --------
                # (depends only on the gathered V, runs on gpsimd + spare
                #  DMA while the tensor engine is busy with T)
                ew_rs = []
                with (
                    tc.tile_pool(name="vew", bufs=2) as vewp,
                    tc.tile_pool(name="cvew", bufs=2) as cvewp,
                    tc.tile_pool(name="ewp", bufs=1) as ewp0,
                ):
                    for ds in range(NDS):
                        vt = vewp.tile([P, KB, 512], bf16, tag="v",
                                       name=f"vew_{ds}")
                        nc.sync.dma_start(vt[:], v_gth[ds % NDSH, ds // NDSH])
                        cvt = cvewp.tile([P, KB, 512], bf16, tag="cv",
                                         name=f"cvew_{ds}")
                        nc.sync.dma_start(cvt[:], cvT[ds])
                        KH = KB // 2
                        t2a = ewp0.tile([P, KH, 512], bf16, tag="t2a",
                                        name=f"ewt2a_{ds}")
                        nc.vector.tensor_mul(t2a[:], vt[:, 0:KH, :],
                                             cvt[:, 0:KH, :])
                        t2b = ewp0.tile([P, KH, 512], bf16, tag="t2b",
                                        name=f"ewt2b_{ds}")
                        nc.vector.tensor_mul(t2b[:], vt[:, KH:KB, :],
                                             cvt[:, KH:KB, :])
                        nc.vector.tensor_add(t2a[:], t2a[:], t2b[:])
                        t3 = ewp0.tile([P, KB // 4, 512], bf16, tag="t3",
                                       name=f"ewt3_{ds}")
                        nc.vector.tensor_add(t3[:], t2a[:, 0:KB // 4, :],
                                             t2a[:, KB // 4:KH, :])
                        # lives in the long-lived ptp pool: read by O phase
                        ew_r = ptp.tile([P, 512], f32r, tag=f"ewr{ds}",
                                        name=f"ewr_{ds}")
                        nc.vector.tensor_add(ew_r[:], t3[:, 0, :], t3[:, 1, :])
                        ew_rs.append(ew_r)

                # ------- L: logits (bf16), softmax, P^T -------------------
                with (
                    tc.tile_pool(name="pp", bufs=2) as pp,
                    tc.tile_pool(name="sm", bufs=16) as smp,
                    tc.tile_pool(name="ps3", bufs=8, space="PSUM") as ps3,
                ):
                    lg = [[ps3.tile([P, NL], f32, tag="ps", name=f"lg_{qs}_{kh}")
                           for kh in range(KN)] for qs in range(QS)]
                    # qs-outer so lg[0] finishes early and its softmax +
                    # transposes overlap the remaining logits matmuls
                    for qs in range(QS):
                        for db in range(DB):
                            for kh in range(KN):
                                nc.tensor.matmul(
                                    lg[qs][kh][:],
                                    tt[:, db, qs * P:(qs + 1) * P],
                                    xte(db)[:, kh * NL:(kh + 1) * NL],
                                    start=(db == 0), stop=(db == DB - 1))
                    # P^T holds UNNORMALIZED exp; 1/z is applied as the
                    # activation scale on the final PSUM->SBUF copy, so the
                    # transposes start right after exp (no vector chain in
                    # the critical path).
                    rs = []
                    for qs in range(QS):
                        p_t = pp.tile([P, L], bf16, tag="p", name=f"p_{qs}")
                        zs = []
                        for kh in range(KN):
                            z = smp.tile([P, 1], f32, tag="sm",
                                         name=f"z_{qs}_{kh}")
                            nc.scalar.activation(
                                p_t[:, kh * NL:(kh + 1) * NL], lg[qs][kh][:],
                                Exp, scale=scale, accum_out=z[:])
                            zs.append(z)
                        for kb in range(KB):
                            pst = ps3.tile([P, P], bf16, tag="ps",
                                           name=f"pst_{qs}_{kb}")
                            nc.tensor.transpose(
                                pst[:], p_t[:, kb * P:(kb + 1) * P], ident[:])
                            nc.vector.tensor_copy(
                                pt_sb[:, kb, qs * P:(qs + 1) * P], pst[:])
                        zfull = zs[0]
                        for kh in range(1, KN):
                            z2 = smp.tile([P, 1], f32, tag="sm",
                                          name=f"zz_{qs}_{kh}")
                            nc.vector.tensor_add(z2[:], zfull[:], zs[kh][:])
                            zfull = z2
                        r = ptp.tile([P, 1], f32, tag=f"r{qs}",
                                     name=f"r_{qs}")
                        nc.vector.reciprocal(r[:], zfull[:])
                        rs.append(r)

                # ------- O: out = P^T.T @ V0 + ones*colsum(cvw.T*V0) ------
                with (
                    tc.tile_pool(name="vl", bufs=3) as vlp,
                    tc.tile_pool(name="ob", bufs=4) as obp,
                    tc.tile_pool(name="psO", bufs=8, space="PSUM") as psO,
                ):
                    for ds in range(NDS):
                        pso = [psO.tile([P, 512], f32, tag="po",
                                        name=f"pso_{ds}_{qs}")
                               for qs in range(QS)]
                        vt = vlp.tile([P, KB, 512], bf16, tag="v",
                                      name=f"vl_{ds}")
                        nc.sync.dma_start(vt[:], v_gth[ds % NDSH, ds // NDSH])
                        for kb in range(KB):
                            for qs in range(QS):
                                nc.tensor.matmul(
                                    pso[qs][:],
                                    pt_sb[:, kb, qs * P:(qs + 1) * P],
                                    vt[:, kb, :],
                                    start=(kb == 0), stop=(kb == KB - 1))
                        # conv term: colsum(ew_r) broadcast to all 128 rows
                        ec_ps = psO.tile([P, 512], f32, tag="po",
                                         name=f"ec_{ds}")
                        nc.tensor.matmul(ec_ps[:], ones[:], ew_rs[ds][:],
                                         start=True, stop=True)
                        ecb = obp.tile([P, 512], f32, tag="ec",
                                       name=f"ecb_{ds}")
                        nc.vector.tensor_copy(ecb[:], ec_ps[:])
                        for qs in range(QS):
                            osb = obp.tile([P, 512], f32, tag="o",
                                           name=f"o_{ds}_{qs}")
                            nc.scalar.activation(osb[:], pso[qs][:], Ident,
                                                 scale=rs[qs][:])
                            nc.vector.tensor_add(osb[:], osb[:], ecb[:])
                            nc.sync.dma_start(
                                out[qs * P:(qs + 1) * P,
                                    ds * 512:(ds + 1) * 512], osb[:])
    nc.compile()
    return nc


# ----------------------------------------------------------------------
# Host side
# ----------------------------------------------------------------------

_CACHE = {}


def _get_nc(key, cfg):
    if key not in _CACHE:
        _CACHE[key] = build(cfg)
    return _CACHE[key]


def _bf16(a):
    import ml_dtypes
    return np.ascontiguousarray(a, dtype=ml_dtypes.bfloat16)


def _prep_shared(cfg, wq, bq, wk, wv, cvw):
    EB, DGN, NDS, KB, DB = (cfg["EB"], cfg["DGN"], cfg["NDS"],
                            cfg["KB"], cfg["DB"])
    ECW, ECN = cfg["ECW"], cfg["ECN"]
    wq = np.asarray(wq, np.float32)
    wk = np.asarray(wk, np.float32)
    A = wq.T @ wk                       # [e, d]
    u = np.asarray(bq, np.float32) @ wk  # [d]
    Ah = _bf16(A.reshape(EB, P, DGN, 512).transpose(0, 2, 1, 3))
    wvTh = _bf16(np.asarray(wv, np.float32).T
                 .reshape(ECN, ECW, P, NDS, 512).transpose(3, 0, 2, 1, 4))
    cvTh = _bf16(np.asarray(cvw, np.float32).T
                 .reshape(KB, P, NDS, 512).transpose(2, 1, 0, 3))
    uh = np.ascontiguousarray(u.reshape(DB, P).T, dtype=np.float32)
    return Ah, wvTh, cvTh, uh


def make_in_maps(cfg, x, wq, bq, wk, wv, cvw):
    QH, NDSH = cfg["QH"], cfg["NDSH"]
    B = x.shape[0]
    n_cores = B * (cfg["L"] // QH)
    Ah, wvTh, cvTh, uh = _prep_shared(cfg, wq, bq, wk, wv, cvw)
    wvT_halves = [np.ascontiguousarray(wvTh[:NDSH]),
                  np.ascontiguousarray(wvTh[NDSH:])]
    ones_h = np.ones((P, P), dtype=np.float32)
    EB, L = cfg["EB"], cfg["L"]
    in_maps = []
    for c in range(n_cores):
        b, ch = c // 2, c % 2
        # [P, EB, L] SBUF layout so the device loads x in 4 big DMAs
        xbT = np.asarray(x[b], np.float32).T.reshape(EB, P, L)
        xbT = xbT.transpose(1, 0, 2)
        in_maps.append(dict(
            xT=_bf16(xbT),
            xTq=_bf16(xbT[:, :, ch * QH:(ch + 1) * QH]),
            Ah=Ah, wvT=wvT_halves[ch], cvT=cvTh, uh=uh, onesd=ones_h,
        ))
    return in_maps, n_cores


def host_add_vec(bv, cvw, cvb):
    bv = np.asarray(bv, np.float32)
    cvw = np.asarray(cvw, np.float32)
    cvb = np.asarray(cvb, np.float32)
    return (bv * (1.0 + cvw.sum(axis=1)) + cvb).astype(np.float32)


def _gather(cfg, results, B, bv, cvw, cvb):
    QH, L, D = cfg["QH"], cfg["L"], cfg["D"]
    out = np.empty((B, L, D), dtype=np.float32)
    for c in range(2 * B):
        b, ch = c // 2, c % 2
        out[b, ch * QH:(ch + 1) * QH, :] = results[c]["out"]
    out += host_add_vec(bv, cvw, cvb)[None, None, :]
    return out


def kernel(x, wq, bq, wk, bk, wv, bv, ckw, ckb, cvw, cvb):
    """Full-input entry point. bk/ckw/ckb are mathematically dead (see top)."""
    from concourse.bass_utils import run_bass_kernel_spmd

    x = np.asarray(x, dtype=np.float32)
    cfg = _cfg(4096, 1024, 512)
    in_maps, n_cores = make_in_maps(cfg, x, wq, bq, wk, wv, cvw)
    nc = _get_nc(("full", 4096, 1024, 512), cfg)
    res = run_bass_kernel_spmd(nc, in_maps, core_ids=list(range(n_cores)))
    return _gather(cfg, res.results, x.shape[0], bv, cvw, cvb)
